# revision 13
# baseline (speedup 1.0000x reference)
"""Trainium2 Bass kernel for nn_Matrix_63952063037710 (GNN message passing).

Math (reference):
    x    = inp @ Wpre.T + bpre                      # [B, dim]
    gate = relu(life)                               # [num, num]
    Wg   = gate[:,:,None,None] * W                  # [num, num, e, d]
    bias = einsum('ij,ijd->jd', gate, b)            # [num, dim]
    m0   = [x, 0, ..., 0]                           # [num, B, dim]
    repeat steps: new[j] = sum_i m[i] @ Wg[i,j].T + bias[j]
    out  = m[num-1] @ Wpost.T + bpost               # [B, out_c]

Strategy: data-parallel over 8 NeuronCores (B=4096 -> 512 rows/core).
State kept transposed in SBUF as [dim=128 partitions, 512 batch] tiles.
Per (i,j) edge: one matmul with stationary lhsT = Wg[i,j].T [d,e] and
moving rhs = m[i].T [d, 512], accumulated over i in a PSUM bank (fp32).
Bias-add fused into the PSUM->SBUF evacuation on ScalarE (Identity act).
Matmul dtype float32r: full-rate (1 cyc/row at N=512) with ~tf32-like
precision; state stored in full fp32 (bitcast to f32r for the PE).
Step 1 only needs i=0 (all other states are zero).
"""

import numpy as np
import ml_dtypes

import concourse.bass as bass
import concourse.tile as tile
from concourse import bacc, mybir
from concourse.bass_utils import run_bass_kernel_spmd

B, IN_C, OUT_C, NUM, DIM = 4096, 512, 512, 16, 128
NCORES = 8
BL = B // NCORES          # 512 batch rows per core
F32 = mybir.dt.float32

# variant: "f32r" (default) or "bf16"
VARIANT = "f32r"


def _mm_dt(variant):
    return mybir.dt.float32r if variant == "f32r" else mybir.dt.bfloat16


def _np_dt(variant):
    return np.float32 if variant == "f32r" else ml_dtypes.bfloat16


def build(steps, variant=VARIANT, n_wg_dma=16):
    """Build the Bacc program for one core (SPMD-identical across cores)."""
    assert steps >= 1
    mmdt = _mm_dt(variant)
    # state tiles carry the matmul dtype directly: the BIR verifier requires
    # fp32r matmul operands to be *produced* rounded to fp32r (ACT does it)
    sdt = mmdt

    nc = bacc.Bacc("TRN2", target_bir_lowering=False, debug=False,
                   num_devices=NCORES)
    xT_d = nc.dram_tensor("xT", [4, 128, BL], mmdt, kind="ExternalInput").ap()
    wpre_d = nc.dram_tensor("wpreT", [4, 128, 128], mmdt, kind="ExternalInput").ap()
    bpre_d = nc.dram_tensor("bpre", [128, 1], F32, kind="ExternalInput").ap()
    # wg host layout: [i, d, j*e] so each chunk-i DMA is a plain 2D
    # contiguous-per-partition transfer with an exact one-tile dependency
    wg_d = nc.dram_tensor("wg", [NUM, 128, NUM * 128], mmdt, kind="ExternalInput").ap()
    bias_d = nc.dram_tensor("biasT", [128, NUM], F32, kind="ExternalInput").ap()
    wpost_d = nc.dram_tensor("wpostT", [128, OUT_C], mmdt, kind="ExternalInput").ap()
    bpost_d = nc.dram_tensor("bpostT", [128, 4], F32, kind="ExternalInput").ap()
    o_d = nc.dram_tensor("o", [4, 128, BL], F32, kind="ExternalOutput").ap()

    with tile.TileContext(nc) as tc:
        with tc.tile_pool(name="wgp", bufs=1) as wgp, \
             tc.tile_pool(name="statep", bufs=1) as statep, \
             tc.tile_pool(name="constp", bufs=1) as constp, \
             tc.tile_pool(name="workp", bufs=4) as workp, \
             tc.tile_pool(name="psp", bufs=8, space="PSUM") as psp:

            # ---- small inputs first: pre-layer + consts can start at ~5us
            xts = []
            wpts = []
            for c in range(4):
                xt = workp.tile([128, BL], mmdt, tag="x", name=f"xt{c}")
                nc.sync.dma_start(xt[:], xT_d[c])
                xts.append(xt)
                wpt = workp.tile([128, 128], mmdt, tag="wp", name=f"wpt{c}")
                nc.sync.dma_start(wpt[:], wpre_d[c])
                wpts.append(wpt)
            biasT = constp.tile([128, NUM], F32, name="biasT")
            nc.sync.dma_start(biasT[:], bias_d)
            bpre_t = constp.tile([128, 1], F32, name="bpre_t")
            nc.sync.dma_start(bpre_t[:], bpre_d)

            # ---- edge weights: one tile per source i (16 x [128, 16*128]).
            # scalar's HWDGE queue is otherwise empty -> it carries the
            # early chunks (0-9, incl. chunk 0 that step 1 needs ~10us in);
            # sync carries the tail chunks behind the small input DMAs.
            wgt = []
            for i in range(NUM):
                w = wgp.tile([128, NUM * 128], mmdt, tag=f"wg{i}",
                             name=f"wgt{i}")
                eng = nc.scalar if i < 10 else nc.sync
                eng.dma_start(w[:], wg_d[i])
                wgt.append(w)

            # post-layer constants are only needed at the very end
            bpost_t = constp.tile([128, 4], F32, name="bpost_t")
            nc.sync.dma_start(bpost_t[:], bpost_d)
            wpost_t = constp.tile([128, OUT_C], mmdt, name="wpost_t")
            nc.sync.dma_start(wpost_t[:], wpost_d)

            def wslice(i, j):
                return wgt[i][:, j * 128:(j + 1) * 128]

            stateA = statep.tile([128, NUM * BL], sdt, name="stateA")
            stateB = statep.tile([128, NUM * BL], sdt, name="stateB")

            ident = mybir.ActivationFunctionType.Identity

            # ---- PE warm-up: a throwaway accumulation group on scratch data
            # with no DMA dependency, so the HAM clock-gate reaches 8/8
            # before the real matmuls start (~3.4us of sustained activity)
            scratch = constp.tile([128, 128], mmdt, name="scratch")
            nc.gpsimd.memset(scratch[:].bitcast(F32) if mmdt != F32
                             else scratch[:], 0)
            warm_ps = psp.tile([128, BL], F32, tag="ps", name="warm_ps")
            for w in range(12):
                nc.tensor.matmul(warm_ps[:], scratch[:],
                                 stateA[:, 0:BL], start=(w == 0),
                                 stop=(w == 11), skip_group_check=True)

            # ---- pre layer: x.T = Wpre @ inp.T  (+bpre) -> stateA[0] ----
            ps = psp.tile([128, BL], F32, tag="ps", name="ps_pre")
            for c in range(4):
                nc.tensor.matmul(ps[:], wpts[c][:], xts[c][:],
                                 start=(c == 0), stop=(c == 3))
            nc.scalar.activation(stateA[:, 0:BL], ps[:], ident,
                                 bias=bpre_t[:, 0:1])

            # ---- message-passing steps ----
            cur, nxt = stateA, stateB

            # step 1: only i=0 is nonzero (and only j=15 matters if it is
            # also the last step)
            for j in ([NUM - 1] if steps == 1 else range(NUM)):
                ps = psp.tile([128, BL], F32, tag="ps", name=f"ps_s1_{j}")
                nc.tensor.matmul(ps[:], wslice(0, j),
                                 cur[:, 0:BL], start=True, stop=True)
                nc.scalar.activation(nxt[:, j * BL:(j + 1) * BL], ps[:], ident,
                                     bias=biasT[:, j:j + 1])
            cur, nxt = nxt, cur

            # steps 2..S: full 16x16 contraction.
            # The last step only needs j=15 (the post layer reads m[15] alone).
            for t in range(1, steps):
                js = [NUM - 1] if t == steps - 1 else list(range(NUM))
                if t == 1 and len(js) == NUM:
                    # first full step overlaps the streaming weight DMA:
                    # i-outer across banks of 8 so the PE consumes weight
                    # chunk i as soon as it lands instead of stalling on
                    # the last chunk inside one j-group.
                    for half in range(2):
                        jh = js[half * 8:(half + 1) * 8]
                        pss = {j: psp.tile([128, BL], F32, tag="ps",
                                           name=f"ps_{t}_{j}") for j in jh}
                        for i in range(NUM):
                            for j in jh:
                                nc.tensor.matmul(
                                    pss[j][:], wslice(i, j),
                                    cur[:, i * BL:(i + 1) * BL],
                                    start=(i == 0), stop=(i == NUM - 1))
                        for j in jh:
                            nc.scalar.activation(
                                nxt[:, j * BL:(j + 1) * BL], pss[j][:],
                                ident, bias=biasT[:, j:j + 1])
                else:
                    for j in js:
                        ps = psp.tile([128, BL], F32, tag="ps",
                                      name=f"ps_{t}_{j}")
                        for i in range(NUM):
                            nc.tensor.matmul(ps[:], wslice(i, j),
                                             cur[:, i * BL:(i + 1) * BL],
                                             start=(i == 0), stop=(i == NUM - 1))
                        nc.scalar.activation(nxt[:, j * BL:(j + 1) * BL], ps[:],
                                             ident, bias=biasT[:, j:j + 1])
                cur, nxt = nxt, cur

            # ---- post layer: out.T = Wpost @ m[15].T (+bpost) ----
            last = cur[:, (NUM - 1) * BL:NUM * BL]
            for c in range(4):
                ps = psp.tile([128, BL], F32, tag="ps", name=f"ps_post{c}")
                nc.tensor.matmul(ps[:], wpost_t[:, c * 128:(c + 1) * 128],
                                 last, start=True, stop=True)
                ot = workp.tile([128, BL], F32, tag="x", name=f"ot{c}")
                nc.scalar.activation(ot[:], ps[:], ident,
                                     bias=bpost_t[:, c:c + 1])
                nc.sync.dma_start(o_d[c], ot[:])

    nc.compile()
    return nc


def make_in_maps(inp, Wpre, bpre, W, b, life, Wpost, bpost, variant=VARIANT):
    npdt = _np_dt(variant)
    f32 = np.float32
    gate = np.where(life > 0, life, 0.0).astype(f32)
    Wg = (gate[:, :, None, None] * W.astype(f32))
    wg = np.ascontiguousarray(
        Wg.transpose(0, 3, 1, 2).reshape(NUM, DIM, NUM * DIM)).astype(npdt)
    biasT = np.ascontiguousarray(
        np.einsum('ij,ijd->jd', gate, b.astype(f32)).T).astype(f32)
    wpreT = np.ascontiguousarray(Wpre.astype(f32).T).reshape(4, 128, 128).astype(npdt)
    bpre_c = np.ascontiguousarray(bpre.astype(f32).reshape(128, 1))
    wpostT = np.ascontiguousarray(Wpost.astype(f32).T).astype(npdt)
    bpostT = np.ascontiguousarray(bpost.astype(f32).reshape(4, 128).T)

    shared = {"wpreT": wpreT, "bpre": bpre_c, "wg": wg, "biasT": biasT,
              "wpostT": wpostT, "bpostT": bpostT}
    in_maps = []
    for k in range(NCORES):
        xT = np.ascontiguousarray(
            inp[k * BL:(k + 1) * BL].astype(f32).T).reshape(4, 128, BL).astype(npdt)
        in_maps.append({"xT": xT, **shared})
    return in_maps


def assemble(results):
    out = np.empty((B, OUT_C), np.float32)
    for k in range(NCORES):
        out[k * BL:(k + 1) * BL] = results[k]["o"].reshape(OUT_C, BL).T
    return out


_CACHE = {}


def kernel(inp, Wpre, bpre, W, b, life, Wpost, bpost, steps):
    steps = int(steps)
    if steps == 0:
        # m[15] stays zero -> output is just the broadcast post bias
        return np.broadcast_to(bpost.astype(np.float32), (B, OUT_C)).copy()
    key = (steps, VARIANT)
    if key not in _CACHE:
        _CACHE[key] = build(steps, VARIANT)
    nc = _CACHE[key]
    in_maps = make_in_maps(inp, Wpre, bpre, W, b, life, Wpost, bpost, VARIANT)
    res = run_bass_kernel_spmd(nc, in_maps, core_ids=list(range(NCORES)))
    return assemble(res.results)


# revision 15
# speedup vs baseline: 1.0135x; 1.0135x over previous
"""Trainium2 Bass kernel for nn_Matrix_63952063037710 (GNN message passing).

Math (reference):
    x    = inp @ Wpre.T + bpre                      # [B, dim]
    gate = relu(life)                               # [num, num]
    Wg   = gate[:,:,None,None] * W                  # [num, num, e, d]
    bias = einsum('ij,ijd->jd', gate, b)            # [num, dim]
    m0   = [x, 0, ..., 0]                           # [num, B, dim]
    repeat steps: new[j] = sum_i m[i] @ Wg[i,j].T + bias[j]
    out  = m[num-1] @ Wpost.T + bpost               # [B, out_c]

Strategy: data-parallel over 8 NeuronCores (B=4096 -> 512 rows/core).
State kept transposed in SBUF as [dim=128 partitions, 512 batch] tiles.
Per (i,j) edge: one matmul with stationary lhsT = Wg[i,j].T [d,e] and
moving rhs = m[i].T [d, 512], accumulated over i in a PSUM bank (fp32).
Bias-add fused into the PSUM->SBUF evacuation on ScalarE (Identity act).
Matmul dtype float32r: full-rate (1 cyc/row at N=512) with ~tf32-like
precision; state stored in full fp32 (bitcast to f32r for the PE).
Step 1 only needs i=0 (all other states are zero).
"""

import numpy as np
import ml_dtypes

import concourse.bass as bass
import concourse.tile as tile
from concourse import bacc, mybir
from concourse.bass_utils import run_bass_kernel_spmd

B, IN_C, OUT_C, NUM, DIM = 4096, 512, 512, 16, 128
NCORES = 8
BL = B // NCORES          # 512 batch rows per core
F32 = mybir.dt.float32

# variant: "f32r" (default) or "bf16"
VARIANT = "f32r"


def _mm_dt(variant):
    return mybir.dt.float32r if variant == "f32r" else mybir.dt.bfloat16


def _np_dt(variant):
    return np.float32 if variant == "f32r" else ml_dtypes.bfloat16


def build(steps, variant=VARIANT, n_wg_dma=16):
    """Build the Bacc program for one core (SPMD-identical across cores)."""
    assert steps >= 1
    mmdt = _mm_dt(variant)
    # state tiles carry the matmul dtype directly: the BIR verifier requires
    # fp32r matmul operands to be *produced* rounded to fp32r (ACT does it)
    sdt = mmdt

    nc = bacc.Bacc("TRN2", target_bir_lowering=False, debug=False,
                   num_devices=NCORES)
    xT_d = nc.dram_tensor("xT", [4, 128, BL], mmdt, kind="ExternalInput").ap()
    wpre_d = nc.dram_tensor("wpreT", [4, 128, 128], mmdt, kind="ExternalInput").ap()
    bpre_d = nc.dram_tensor("bpre", [128, 1], F32, kind="ExternalInput").ap()
    # wg host layout: [i, d, j*e] so each chunk-i DMA is a plain 2D
    # contiguous-per-partition transfer with an exact one-tile dependency
    wg_d = nc.dram_tensor("wg", [NUM, 128, NUM * 128], mmdt, kind="ExternalInput").ap()
    bias_d = nc.dram_tensor("biasT", [128, NUM], F32, kind="ExternalInput").ap()
    wpost_d = nc.dram_tensor("wpostT", [128, OUT_C], mmdt, kind="ExternalInput").ap()
    bpost_d = nc.dram_tensor("bpostT", [128, 4], F32, kind="ExternalInput").ap()
    o_d = nc.dram_tensor("o", [4, 128, BL], F32, kind="ExternalOutput").ap()

    with tile.TileContext(nc) as tc:
        with tc.tile_pool(name="wgp", bufs=1) as wgp, \
             tc.tile_pool(name="statep", bufs=1) as statep, \
             tc.tile_pool(name="constp", bufs=1) as constp, \
             tc.tile_pool(name="workp", bufs=4) as workp, \
             tc.tile_pool(name="psp", bufs=8, space="PSUM") as psp:

            # ---- small inputs first: pre-layer + consts can start at ~5us
            xts = []
            wpts = []
            for c in range(4):
                xt = workp.tile([128, BL], mmdt, tag="x", name=f"xt{c}")
                nc.sync.dma_start(xt[:], xT_d[c])
                xts.append(xt)
                wpt = workp.tile([128, 128], mmdt, tag="wp", name=f"wpt{c}")
                nc.sync.dma_start(wpt[:], wpre_d[c])
                wpts.append(wpt)
            biasT = constp.tile([128, NUM], F32, name="biasT")
            nc.sync.dma_start(biasT[:], bias_d)
            bpre_t = constp.tile([128, 1], F32, name="bpre_t")
            nc.sync.dma_start(bpre_t[:], bpre_d)

            # ---- edge weights: one tile per source i (16 x [128, 16*128]).
            # scalar's HWDGE queue is otherwise empty -> it carries the
            # early chunks (0-9, incl. chunk 0 that step 1 needs ~10us in);
            # sync carries the tail chunks behind the small input DMAs.
            wgt = []
            for i in range(NUM):
                w = wgp.tile([128, NUM * 128], mmdt, tag=f"wg{i}",
                             name=f"wgt{i}")
                eng = nc.scalar if i % 2 == 0 else nc.sync
                eng.dma_start(w[:], wg_d[i])
                wgt.append(w)

            # post-layer constants are only needed at the very end
            bpost_t = constp.tile([128, 4], F32, name="bpost_t")
            nc.sync.dma_start(bpost_t[:], bpost_d)
            wpost_t = constp.tile([128, OUT_C], mmdt, name="wpost_t")
            nc.sync.dma_start(wpost_t[:], wpost_d)

            def wslice(i, j):
                return wgt[i][:, j * 128:(j + 1) * 128]

            stateA = statep.tile([128, NUM * BL], sdt, name="stateA")
            stateB = statep.tile([128, NUM * BL], sdt, name="stateB")

            ident = mybir.ActivationFunctionType.Identity

            # ---- PE warm-up: a throwaway accumulation group on scratch data
            # with no DMA dependency, so the HAM clock-gate reaches 8/8
            # before the real matmuls start (~3.4us of sustained activity)
            scratch = constp.tile([128, BL], mmdt, name="scratch")
            nc.gpsimd.memset(scratch[:].bitcast(F32) if mmdt != F32
                             else scratch[:], 0)
            warm_ps = psp.tile([128, BL], F32, tag="ps", name="warm_ps")
            for w in range(12):
                nc.tensor.matmul(warm_ps[:], scratch[:, 0:128],
                                 scratch[:], start=(w == 0),
                                 stop=(w == 11), skip_group_check=True)

            # ---- pre layer: x.T = Wpre @ inp.T  (+bpre) -> stateA[0] ----
            ps = psp.tile([128, BL], F32, tag="ps", name="ps_pre")
            for c in range(4):
                nc.tensor.matmul(ps[:], wpts[c][:], xts[c][:],
                                 start=(c == 0), stop=(c == 3))
            nc.scalar.activation(stateA[:, 0:BL], ps[:], ident,
                                 bias=bpre_t[:, 0:1])

            # ---- message-passing steps ----
            cur, nxt = stateA, stateB

            # step 1: only i=0 is nonzero (and only j=15 matters if it is
            # also the last step)
            for j in ([NUM - 1] if steps == 1 else range(NUM)):
                ps = psp.tile([128, BL], F32, tag="ps", name=f"ps_s1_{j}")
                nc.tensor.matmul(ps[:], wslice(0, j),
                                 cur[:, 0:BL], start=True, stop=True)
                nc.scalar.activation(nxt[:, j * BL:(j + 1) * BL], ps[:], ident,
                                     bias=biasT[:, j:j + 1])
            cur, nxt = nxt, cur

            # steps 2..S: full 16x16 contraction.
            # The last step only needs j=15 (the post layer reads m[15] alone).
            for t in range(1, steps):
                js = [NUM - 1] if t == steps - 1 else list(range(NUM))
                if t == 1 and len(js) == NUM:
                    # first full step overlaps the streaming weight DMA:
                    # i-outer across banks of 8 so the PE consumes weight
                    # chunk i as soon as it lands instead of stalling on
                    # the last chunk inside one j-group.
                    for half in range(2):
                        jh = js[half * 8:(half + 1) * 8]
                        pss = {j: psp.tile([128, BL], F32, tag="ps",
                                           name=f"ps_{t}_{j}") for j in jh}
                        for i in range(NUM):
                            for j in jh:
                                nc.tensor.matmul(
                                    pss[j][:], wslice(i, j),
                                    cur[:, i * BL:(i + 1) * BL],
                                    start=(i == 0), stop=(i == NUM - 1))
                        for j in jh:
                            nc.scalar.activation(
                                nxt[:, j * BL:(j + 1) * BL], pss[j][:],
                                ident, bias=biasT[:, j:j + 1])
                else:
                    for j in js:
                        ps = psp.tile([128, BL], F32, tag="ps",
                                      name=f"ps_{t}_{j}")
                        for i in range(NUM):
                            nc.tensor.matmul(ps[:], wslice(i, j),
                                             cur[:, i * BL:(i + 1) * BL],
                                             start=(i == 0), stop=(i == NUM - 1))
                        nc.scalar.activation(nxt[:, j * BL:(j + 1) * BL], ps[:],
                                             ident, bias=biasT[:, j:j + 1])
                cur, nxt = nxt, cur

            # ---- post layer: out.T = Wpost @ m[15].T (+bpost) ----
            last = cur[:, (NUM - 1) * BL:NUM * BL]
            for c in range(4):
                ps = psp.tile([128, BL], F32, tag="ps", name=f"ps_post{c}")
                nc.tensor.matmul(ps[:], wpost_t[:, c * 128:(c + 1) * 128],
                                 last, start=True, stop=True)
                ot = workp.tile([128, BL], F32, tag="x", name=f"ot{c}")
                nc.scalar.activation(ot[:], ps[:], ident,
                                     bias=bpost_t[:, c:c + 1])
                nc.sync.dma_start(o_d[c], ot[:])

    nc.compile()
    return nc


def make_in_maps(inp, Wpre, bpre, W, b, life, Wpost, bpost, variant=VARIANT):
    npdt = _np_dt(variant)
    f32 = np.float32
    gate = np.where(life > 0, life, 0.0).astype(f32)
    Wg = (gate[:, :, None, None] * W.astype(f32))
    wg = np.ascontiguousarray(
        Wg.transpose(0, 3, 1, 2).reshape(NUM, DIM, NUM * DIM)).astype(npdt)
    biasT = np.ascontiguousarray(
        np.einsum('ij,ijd->jd', gate, b.astype(f32)).T).astype(f32)
    wpreT = np.ascontiguousarray(Wpre.astype(f32).T).reshape(4, 128, 128).astype(npdt)
    bpre_c = np.ascontiguousarray(bpre.astype(f32).reshape(128, 1))
    wpostT = np.ascontiguousarray(Wpost.astype(f32).T).astype(npdt)
    bpostT = np.ascontiguousarray(bpost.astype(f32).reshape(4, 128).T)

    shared = {"wpreT": wpreT, "bpre": bpre_c, "wg": wg, "biasT": biasT,
              "wpostT": wpostT, "bpostT": bpostT}
    in_maps = []
    for k in range(NCORES):
        xT = np.ascontiguousarray(
            inp[k * BL:(k + 1) * BL].astype(f32).T).reshape(4, 128, BL).astype(npdt)
        in_maps.append({"xT": xT, **shared})
    return in_maps


def assemble(results):
    out = np.empty((B, OUT_C), np.float32)
    for k in range(NCORES):
        out[k * BL:(k + 1) * BL] = results[k]["o"].reshape(OUT_C, BL).T
    return out


_CACHE = {}


def kernel(inp, Wpre, bpre, W, b, life, Wpost, bpost, steps):
    steps = int(steps)
    if steps == 0:
        # m[15] stays zero -> output is just the broadcast post bias
        return np.broadcast_to(bpost.astype(np.float32), (B, OUT_C)).copy()
    key = (steps, VARIANT)
    if key not in _CACHE:
        _CACHE[key] = build(steps, VARIANT)
    nc = _CACHE[key]
    in_maps = make_in_maps(inp, Wpre, bpre, W, b, life, Wpost, bpost, VARIANT)
    res = run_bass_kernel_spmd(nc, in_maps, core_ids=list(range(NCORES)))
    return assemble(res.results)


# revision 16
# speedup vs baseline: 1.0138x; 1.0002x over previous
"""Trainium2 Bass kernel for nn_Matrix_63952063037710 (GNN message passing).

Math (reference):
    x    = inp @ Wpre.T + bpre                      # [B, dim]
    gate = relu(life)                               # [num, num]
    Wg   = gate[:,:,None,None] * W                  # [num, num, e, d]
    bias = einsum('ij,ijd->jd', gate, b)            # [num, dim]
    m0   = [x, 0, ..., 0]                           # [num, B, dim]
    repeat steps: new[j] = sum_i m[i] @ Wg[i,j].T + bias[j]
    out  = m[num-1] @ Wpost.T + bpost               # [B, out_c]

Strategy: data-parallel over 8 NeuronCores (B=4096 -> 512 rows/core).
State kept transposed in SBUF as [dim=128 partitions, 512 batch] tiles.
Per (i,j) edge: one matmul with stationary lhsT = Wg[i,j].T [d,e] and
moving rhs = m[i].T [d, 512], accumulated over i in a PSUM bank (fp32).
Bias-add fused into the PSUM->SBUF evacuation on ScalarE (Identity act).
Matmul dtype float32r: full-rate (1 cyc/row at N=512) with ~tf32-like
precision; state stored in full fp32 (bitcast to f32r for the PE).
Step 1 only needs i=0 (all other states are zero).
"""

import numpy as np
import ml_dtypes

import concourse.bass as bass
import concourse.tile as tile
from concourse import bacc, mybir
from concourse.bass_utils import run_bass_kernel_spmd

B, IN_C, OUT_C, NUM, DIM = 4096, 512, 512, 16, 128
NCORES = 8
BL = B // NCORES          # 512 batch rows per core
F32 = mybir.dt.float32

# variant: "f32r" (default) or "bf16"
VARIANT = "f32r"


def _mm_dt(variant):
    return mybir.dt.float32r if variant == "f32r" else mybir.dt.bfloat16


def _np_dt(variant):
    return np.float32 if variant == "f32r" else ml_dtypes.bfloat16


def build(steps, variant=VARIANT, n_wg_dma=16):
    """Build the Bacc program for one core (SPMD-identical across cores)."""
    assert steps >= 1
    mmdt = _mm_dt(variant)
    # state tiles carry the matmul dtype directly: the BIR verifier requires
    # fp32r matmul operands to be *produced* rounded to fp32r (ACT does it)
    sdt = mmdt

    nc = bacc.Bacc("TRN2", target_bir_lowering=False, debug=False,
                   num_devices=NCORES)
    xT_d = nc.dram_tensor("xT", [4, 128, BL], mmdt, kind="ExternalInput").ap()
    wpre_d = nc.dram_tensor("wpreT", [4, 128, 128], mmdt, kind="ExternalInput").ap()
    bpre_d = nc.dram_tensor("bpre", [128, 1], F32, kind="ExternalInput").ap()
    # wg host layout: [i, d, j*e] so each chunk-i DMA is a plain 2D
    # contiguous-per-partition transfer with an exact one-tile dependency
    wg_d = nc.dram_tensor("wg", [NUM, 128, NUM * 128], mmdt, kind="ExternalInput").ap()
    bias_d = nc.dram_tensor("biasT", [128, NUM], F32, kind="ExternalInput").ap()
    wpost_d = nc.dram_tensor("wpostT", [128, OUT_C], mmdt, kind="ExternalInput").ap()
    bpost_d = nc.dram_tensor("bpostT", [128, 4], F32, kind="ExternalInput").ap()
    o_d = nc.dram_tensor("o", [4, 128, BL], F32, kind="ExternalOutput").ap()

    with tile.TileContext(nc) as tc:
        with tc.tile_pool(name="wgp", bufs=1) as wgp, \
             tc.tile_pool(name="statep", bufs=1) as statep, \
             tc.tile_pool(name="constp", bufs=1) as constp, \
             tc.tile_pool(name="workp", bufs=4) as workp, \
             tc.tile_pool(name="psp", bufs=8, space="PSUM") as psp:

            # ---- small inputs first: pre-layer + consts can start at ~5us
            xts = []
            wpts = []
            for c in range(4):
                xt = workp.tile([128, BL], mmdt, tag="x", name=f"xt{c}")
                nc.sync.dma_start(xt[:], xT_d[c])
                xts.append(xt)
                wpt = workp.tile([128, 128], mmdt, tag="wp", name=f"wpt{c}")
                nc.sync.dma_start(wpt[:], wpre_d[c])
                wpts.append(wpt)
            biasT = constp.tile([128, NUM], F32, name="biasT")
            nc.sync.dma_start(biasT[:], bias_d)
            bpre_t = constp.tile([128, 1], F32, name="bpre_t")
            nc.sync.dma_start(bpre_t[:], bpre_d)

            # ---- edge weights: one tile per source i (16 x [128, 16*128]).
            # scalar's HWDGE queue is otherwise empty -> it carries the
            # early chunks (0-9, incl. chunk 0 that step 1 needs ~10us in);
            # sync carries the tail chunks behind the small input DMAs.
            wgt = []
            for i in range(NUM):
                w = wgp.tile([128, NUM * 128], mmdt, tag=f"wg{i}",
                             name=f"wgt{i}")
                eng = nc.scalar if i % 2 == 0 else nc.sync
                eng.dma_start(w[:], wg_d[i])
                wgt.append(w)

            # post-layer constants are only needed at the very end
            bpost_t = constp.tile([128, 4], F32, name="bpost_t")
            nc.sync.dma_start(bpost_t[:], bpost_d)
            wpost_t = constp.tile([128, OUT_C], mmdt, name="wpost_t")
            nc.sync.dma_start(wpost_t[:], wpost_d)

            def wslice(i, j):
                return wgt[i][:, j * 128:(j + 1) * 128]

            stateA = statep.tile([128, NUM * BL], sdt, name="stateA")
            stateB = statep.tile([128, NUM * BL], sdt, name="stateB")

            ident = mybir.ActivationFunctionType.Identity

            # ---- pre layer: x.T = Wpre @ inp.T  (+bpre) -> stateA[0] ----
            ps = psp.tile([128, BL], F32, tag="ps", name="ps_pre")
            for c in range(4):
                nc.tensor.matmul(ps[:], wpts[c][:], xts[c][:],
                                 start=(c == 0), stop=(c == 3))
            nc.scalar.activation(stateA[:, 0:BL], ps[:], ident,
                                 bias=bpre_t[:, 0:1])

            # ---- message-passing steps ----
            cur, nxt = stateA, stateB

            # step 1: only i=0 is nonzero (and only j=15 matters if it is
            # also the last step)
            for j in ([NUM - 1] if steps == 1 else range(NUM)):
                ps = psp.tile([128, BL], F32, tag="ps", name=f"ps_s1_{j}")
                nc.tensor.matmul(ps[:], wslice(0, j),
                                 cur[:, 0:BL], start=True, stop=True)
                nc.scalar.activation(nxt[:, j * BL:(j + 1) * BL], ps[:], ident,
                                     bias=biasT[:, j:j + 1])
            cur, nxt = nxt, cur

            # steps 2..S: full 16x16 contraction.
            # The last step only needs j=15 (the post layer reads m[15] alone).
            for t in range(1, steps):
                js = [NUM - 1] if t == steps - 1 else list(range(NUM))
                if t == 1 and len(js) == NUM:
                    # first full step overlaps the streaming weight DMA:
                    # i-outer across banks of 8 so the PE consumes weight
                    # chunk i as soon as it lands instead of stalling on
                    # the last chunk inside one j-group.
                    for half in range(2):
                        jh = js[half * 8:(half + 1) * 8]
                        pss = {j: psp.tile([128, BL], F32, tag="ps",
                                           name=f"ps_{t}_{j}") for j in jh}
                        for i in range(NUM):
                            for j in jh:
                                nc.tensor.matmul(
                                    pss[j][:], wslice(i, j),
                                    cur[:, i * BL:(i + 1) * BL],
                                    start=(i == 0), stop=(i == NUM - 1))
                        for j in jh:
                            nc.scalar.activation(
                                nxt[:, j * BL:(j + 1) * BL], pss[j][:],
                                ident, bias=biasT[:, j:j + 1])
                else:
                    for j in js:
                        ps = psp.tile([128, BL], F32, tag="ps",
                                      name=f"ps_{t}_{j}")
                        for i in range(NUM):
                            nc.tensor.matmul(ps[:], wslice(i, j),
                                             cur[:, i * BL:(i + 1) * BL],
                                             start=(i == 0), stop=(i == NUM - 1))
                        nc.scalar.activation(nxt[:, j * BL:(j + 1) * BL], ps[:],
                                             ident, bias=biasT[:, j:j + 1])
                cur, nxt = nxt, cur

            # ---- post layer: out.T = Wpost @ m[15].T (+bpost) ----
            last = cur[:, (NUM - 1) * BL:NUM * BL]
            for c in range(4):
                ps = psp.tile([128, BL], F32, tag="ps", name=f"ps_post{c}")
                nc.tensor.matmul(ps[:], wpost_t[:, c * 128:(c + 1) * 128],
                                 last, start=True, stop=True)
                ot = workp.tile([128, BL], F32, tag="x", name=f"ot{c}")
                nc.scalar.activation(ot[:], ps[:], ident,
                                     bias=bpost_t[:, c:c + 1])
                nc.sync.dma_start(o_d[c], ot[:])

    nc.compile()
    return nc


def make_in_maps(inp, Wpre, bpre, W, b, life, Wpost, bpost, variant=VARIANT):
    npdt = _np_dt(variant)
    f32 = np.float32
    gate = np.where(life > 0, life, 0.0).astype(f32)
    Wg = (gate[:, :, None, None] * W.astype(f32))
    wg = np.ascontiguousarray(
        Wg.transpose(0, 3, 1, 2).reshape(NUM, DIM, NUM * DIM)).astype(npdt)
    biasT = np.ascontiguousarray(
        np.einsum('ij,ijd->jd', gate, b.astype(f32)).T).astype(f32)
    wpreT = np.ascontiguousarray(Wpre.astype(f32).T).reshape(4, 128, 128).astype(npdt)
    bpre_c = np.ascontiguousarray(bpre.astype(f32).reshape(128, 1))
    wpostT = np.ascontiguousarray(Wpost.astype(f32).T).astype(npdt)
    bpostT = np.ascontiguousarray(bpost.astype(f32).reshape(4, 128).T)

    shared = {"wpreT": wpreT, "bpre": bpre_c, "wg": wg, "biasT": biasT,
              "wpostT": wpostT, "bpostT": bpostT}
    in_maps = []
    for k in range(NCORES):
        xT = np.ascontiguousarray(
            inp[k * BL:(k + 1) * BL].astype(f32).T).reshape(4, 128, BL).astype(npdt)
        in_maps.append({"xT": xT, **shared})
    return in_maps


def assemble(results):
    out = np.empty((B, OUT_C), np.float32)
    for k in range(NCORES):
        out[k * BL:(k + 1) * BL] = results[k]["o"].reshape(OUT_C, BL).T
    return out


_CACHE = {}


def kernel(inp, Wpre, bpre, W, b, life, Wpost, bpost, steps):
    steps = int(steps)
    if steps == 0:
        # m[15] stays zero -> output is just the broadcast post bias
        return np.broadcast_to(bpost.astype(np.float32), (B, OUT_C)).copy()
    key = (steps, VARIANT)
    if key not in _CACHE:
        _CACHE[key] = build(steps, VARIANT)
    nc = _CACHE[key]
    in_maps = make_in_maps(inp, Wpre, bpre, W, b, life, Wpost, bpost, VARIANT)
    res = run_bass_kernel_spmd(nc, in_maps, core_ids=list(range(NCORES)))
    return assemble(res.results)
